# revision 2
# baseline (speedup 1.0000x reference)
"""BaselineOrbitals kernel — self-contained, full inputs -> full outputs.

FermiNet-style baseline-orbitals module for B=384 walkers: backflow shift
MLPs over el-el / el-ion pairs, decayed shift applied to el-ion diffs,
gaussian AO molecular orbitals, determinant orbital gather with CI-weight
absorption, and backflow factors.

Execution strategy: pure data parallel over the walker axis (B=384) across
the 8 NeuronCores — every B-leading tensor is sharded on axis 0, all
weights/coefficients are replicated, and the whole module runs as ONE
jitted XLA graph per device (no cross-walker communication). The
broadcast-concat MLP input is factored algebraically:
  concat([x_tiled, pair]) @ W0 == x @ W0[:D] + pair @ W0[D:]
which removes the dominant redundant FLOPs of the reference formulation,
and the determinant orbital gather is rewritten as a one-hot einsum
(computed host-side from the int32 index tensors) so the device graph is
pure matmul/elementwise/transcendental work.

A NumPy fallback implements identical math if no accelerator is reachable.
"""

import numpy as np

B = 384
N_UP = 16; N_DN = 16; N_EL = 32; N_ION = 8
D = 256; P = 32; D_ION = 64; H = 256
N_DETS = 16; N_BASIS = 14; N_AO = N_ION * N_BASIS; N_MO = 64
N_ORB = N_UP + N_DN

# tensors sharded over the walker/batch axis; everything else is replicated
_BATCHED = ('h_el', 'h_el_el', 'h_el_ion', 'diff_el_el', 'dist_el_el',
            'diff_el_ion', 'dist_el_ion')

_DEVICE_FN = None          # (jitted_fn, in_shardings dict, out_sharding)
_DEVICE_INIT_FAILED = False


# ---------------------------------------------------------------- device path
def _build_device_fn():
    import jax, jax.numpy as jnp
    from jax.sharding import Mesh, NamedSharding, PartitionSpec as PS

    devs = jax.devices()
    n = 8 if len(devs) >= 8 else max(d for d in (4, 2, 1) if len(devs) >= d)
    mesh = Mesh(np.array(devs[:n]), ('b',))
    shard_b = NamedSharding(mesh, PS('b'))
    repl = NamedSharding(mesh, PS())

    def fn(v):
        h_el = v['h_el']

        def shift(pair, diff, dist, W0, b0, W1):
            # factored concat-MLP: feat @ W0 = h_el @ W0[:D] + pair @ W0[D:]
            u = h_el @ W0[:D] + b0                       # (b, n_el, H)
            w = pair @ W0[D:]                            # (b, n_el, n_p, H)
            z = jnp.tanh(u[:, :, None, :] + w)
            s = z @ W1                                   # (b, n_el, n_p, 1)
            wgt = s / (1.0 + dist[..., None] ** 3)
            return jnp.sum(wgt * diff, axis=-2)          # (b, n_el, 3)

        s_el = shift(v['h_el_el'], v['diff_el_el'], v['dist_el_el'],
                     v['W_shift_el0'], v['b_shift_el0'], v['W_shift_el1'])
        s_ion = shift(v['h_el_ion'], v['diff_el_ion'], v['dist_el_ion'],
                      v['W_shift_ion0'], v['b_shift_ion0'], v['W_shift_ion1'])
        ls = v['decay_scale'] / jnp.tanh(v['h_ion'] @ v['W_decay']
                                         + v['b_decay'])[..., 0]
        decay = jnp.prod(jnp.tanh((v['dist_el_ion'] / ls) ** 2), axis=-1)
        sh = (s_el + s_ion) * decay[..., None]
        diff_ei = v['diff_el_ion'] + sh[:, :, None, :]
        dist_ei = jnp.sqrt(jnp.sum(diff_ei * diff_ei, axis=-1))

        def mo(dist, coeff):
            ao = jnp.exp(-(dist[..., None] ** 2) * v['alpha'])
            return ao.reshape(ao.shape[:-2] + (N_AO,)) @ coeff

        mo_up = mo(dist_ei[:, :N_UP, :], v['mo_coeff_up'])
        mo_dn = mo(dist_ei[:, N_UP:, :], v['mo_coeff_dn'])
        # orbital gather as one-hot einsum (E_* precomputed host-side)
        sel_up = jnp.einsum('bim,dkm->bdik', mo_up, v['E_up'])
        sel_dn = jnp.einsum('bim,dkm->bdik', mo_dn, v['E_dn'])
        m_up = jnp.concatenate(
            [sel_up, jnp.zeros(sel_up.shape[:-1] + (N_DN,), sel_up.dtype)], -1)
        m_dn = jnp.concatenate(
            [jnp.zeros(sel_dn.shape[:-1] + (N_UP,), sel_dn.dtype), sel_dn], -1)
        m_up = m_up * v['ci_sc']

        def bf(h, W0, b0, W1, b1):
            y = jnp.tanh(h @ W0 + b0) @ W1 + b1
            y = y.reshape(y.shape[:-1] + (N_DETS, N_ORB))
            return jnp.swapaxes(y, -3, -2)

        m_up = m_up * bf(h_el[:, :N_UP, :], v['W_bf_up0'], v['b_bf_up0'],
                         v['W_bf_up1'], v['b_bf_up1'])
        m_dn = m_dn * bf(h_el[:, N_DN:, :], v['W_bf_dn0'], v['b_bf_dn0'],
                         v['W_bf_dn1'], v['b_bf_dn1'])
        return m_up, m_dn

    jf = jax.jit(fn, out_shardings=(shard_b, shard_b))
    return jf, shard_b, repl


def _host_precompute(ins):
    f32 = np.float32
    idx_up = np.asarray(ins['idx_up']).astype(np.int64)
    idx_dn = np.asarray(ins['idx_dn']).astype(np.int64)
    E_up = np.zeros((N_DETS, N_UP, N_MO), f32)
    E_dn = np.zeros((N_DETS, N_DN, N_MO), f32)
    r_up = np.arange(N_UP); r_dn = np.arange(N_DN)
    for d in range(N_DETS):
        E_up[d, r_up, idx_up[d]] = 1.0
        E_dn[d, r_dn, idx_dn[d]] = 1.0
    ciw = np.abs(ins['ci_weights'])[:, None, None] ** f32(1.0 / N_UP)
    sgn = np.concatenate([np.sign(ins['ci_weights'])[:, None, None],
                          np.ones((N_DETS, 1, N_ORB - 1), f32)], axis=-1)
    return E_up, E_dn, (ciw * sgn).astype(f32)


def _run_device(ins):
    global _DEVICE_FN
    import jax
    if _DEVICE_FN is None:
        _DEVICE_FN = _build_device_fn()
    jf, shard_b, repl = _DEVICE_FN

    E_up, E_dn, ci_sc = _host_precompute(ins)
    v = {k: np.ascontiguousarray(np.asarray(ins[k], dtype=np.float32))
         for k in ins if k not in ('idx_up', 'idx_dn')}
    v['E_up'] = E_up; v['E_dn'] = E_dn; v['ci_sc'] = ci_sc
    vp = {k: jax.device_put(v[k], shard_b if k in _BATCHED else repl)
          for k in v}
    m_up, m_dn = jax.block_until_ready(jf(vp))
    return (np.asarray(m_up, dtype=np.float32),
            np.asarray(m_dn, dtype=np.float32))


# ---------------------------------------------------------------- numpy path
def _np_shift(h_el, pair, diff, dist, W0, b0, W1):
    u = h_el @ W0[:D] + b0
    v = pair @ W0[D:]
    z = np.tanh(u[:, :, None, :] + v)
    s = z @ W1
    wgt = s / (1.0 + dist[..., None] ** 3)
    return np.sum(wgt * diff, axis=-2)


def _run_numpy(ins):
    g = lambda k: np.asarray(ins[k]).astype(np.float32)
    h_el = g('h_el')
    s_el = _np_shift(h_el, g('h_el_el'), g('diff_el_el'), g('dist_el_el'),
                     g('W_shift_el0'), g('b_shift_el0'), g('W_shift_el1'))
    s_ion = _np_shift(h_el, g('h_el_ion'), g('diff_el_ion'), g('dist_el_ion'),
                      g('W_shift_ion0'), g('b_shift_ion0'), g('W_shift_ion1'))
    ls = g('decay_scale') / np.tanh(g('h_ion') @ g('W_decay')
                                    + g('b_decay'))[..., 0]
    decay = np.prod(np.tanh((g('dist_el_ion') / ls) ** 2), axis=-1)
    shift = (s_el + s_ion) * decay[..., None]
    diff_ei = g('diff_el_ion') + shift[:, :, None, :]
    dist_ei = np.sqrt(np.sum(diff_ei * diff_ei, axis=-1))

    alpha = g('alpha')

    def mo(dist, coeff):
        ao = np.exp(-(dist[..., None] ** 2) * alpha)
        return ao.reshape(ao.shape[:-2] + (N_AO,)) @ coeff

    mo_up = mo(dist_ei[:, :N_UP, :], g('mo_coeff_up'))
    mo_dn = mo(dist_ei[:, N_UP:, :], g('mo_coeff_dn'))
    idx_up = np.asarray(ins['idx_up'], dtype=np.int64)
    idx_dn = np.asarray(ins['idx_dn'], dtype=np.int64)
    sel_up = np.moveaxis(mo_up[..., idx_up], -2, -3)
    sel_dn = np.moveaxis(mo_dn[..., idx_dn], -2, -3)
    m_up = np.concatenate(
        [sel_up, np.zeros(sel_up.shape[:-1] + (N_DN,), sel_up.dtype)], -1)
    m_dn = np.concatenate(
        [np.zeros(sel_dn.shape[:-1] + (N_UP,), sel_dn.dtype), sel_dn], -1)
    _, _, ci_sc = _host_precompute(ins)
    m_up = m_up * ci_sc

    def bf(h, W0, b0, W1, b1):
        y = np.tanh(h @ W0 + b0) @ W1 + b1
        y = y.reshape(y.shape[:-1] + (N_DETS, N_ORB))
        return np.swapaxes(y, -3, -2)

    m_up = m_up * bf(h_el[:, :N_UP, :], g('W_bf_up0'), g('b_bf_up0'),
                     g('W_bf_up1'), g('b_bf_up1'))
    m_dn = m_dn * bf(h_el[:, N_DN:, :], g('W_bf_dn0'), g('b_bf_dn0'),
                     g('W_bf_dn1'), g('b_bf_dn1'))
    return m_up.astype(np.float32), m_dn.astype(np.float32)


def kernel(**inputs):
    global _DEVICE_INIT_FAILED
    if not _DEVICE_INIT_FAILED:
        try:
            return _run_device(inputs)
        except Exception:
            _DEVICE_INIT_FAILED = True
    return _run_numpy(inputs)


# revision 3
# speedup vs baseline: 1.0590x; 1.0590x over previous
"""BaselineOrbitals kernel — self-contained, full inputs -> full outputs.

FermiNet-style baseline-orbitals module for B=384 walkers: backflow shift
MLPs over el-el / el-ion pairs, decayed shift applied to el-ion diffs,
gaussian AO molecular orbitals, determinant orbital gather with CI-weight
absorption, and backflow factors.

Execution strategy: pure data parallel over the walker axis (B=384) across
the 8 NeuronCores — every B-leading tensor is sharded on axis 0, all
weights/coefficients are replicated, and the whole module runs as ONE
jitted XLA graph per device (no cross-walker communication). The
broadcast-concat MLP input is factored algebraically:
  concat([x_tiled, pair]) @ W0 == x @ W0[:D] + pair @ W0[D:]
which removes the dominant redundant FLOPs of the reference formulation,
and the determinant orbital gather is rewritten as a one-hot einsum
(computed host-side from the int32 index tensors) so the device graph is
pure matmul/elementwise/transcendental work.

A NumPy fallback implements identical math if no accelerator is reachable.
"""

import numpy as np

B = 384
N_UP = 16; N_DN = 16; N_EL = 32; N_ION = 8
D = 256; P = 32; D_ION = 64; H = 256
N_DETS = 16; N_BASIS = 14; N_AO = N_ION * N_BASIS; N_MO = 64
N_ORB = N_UP + N_DN

# tensors sharded over the walker/batch axis; everything else is replicated
_BATCHED = ('h_el', 'h_el_el', 'h_el_ion', 'diff_el_el', 'dist_el_el',
            'diff_el_ion', 'dist_el_ion')

_DEVICE_FN = None          # (jitted_fn, in_shardings dict, out_sharding)
_DEVICE_INIT_FAILED = False


# ---------------------------------------------------------------- device path
def _build_device_fn():
    import jax, jax.numpy as jnp
    from jax.sharding import Mesh, NamedSharding, PartitionSpec as PS

    devs = jax.devices()
    n = 8 if len(devs) >= 8 else max(d for d in (4, 2, 1) if len(devs) >= d)
    mesh = Mesh(np.array(devs[:n]), ('b',))
    shard_b = NamedSharding(mesh, PS('b'))
    repl = NamedSharding(mesh, PS())

    def fn(v):
        h_el = v['h_el']

        def shift(pair, diff, dist, W0, b0, W1):
            # factored concat-MLP: feat @ W0 = h_el @ W0[:D] + pair @ W0[D:]
            u = h_el @ W0[:D] + b0                       # (b, n_el, H)
            w = pair @ W0[D:]                            # (b, n_el, n_p, H)
            z = jnp.tanh(u[:, :, None, :] + w)
            s = z @ W1                                   # (b, n_el, n_p, 1)
            wgt = s / (1.0 + dist[..., None] ** 3)
            return jnp.sum(wgt * diff, axis=-2)          # (b, n_el, 3)

        s_el = shift(v['h_el_el'], v['diff_el_el'], v['dist_el_el'],
                     v['W_shift_el0'], v['b_shift_el0'], v['W_shift_el1'])
        s_ion = shift(v['h_el_ion'], v['diff_el_ion'], v['dist_el_ion'],
                      v['W_shift_ion0'], v['b_shift_ion0'], v['W_shift_ion1'])
        ls = v['decay_scale'] / jnp.tanh(v['h_ion'] @ v['W_decay']
                                         + v['b_decay'])[..., 0]
        decay = jnp.prod(jnp.tanh((v['dist_el_ion'] / ls) ** 2), axis=-1)
        sh = (s_el + s_ion) * decay[..., None]
        diff_ei = v['diff_el_ion'] + sh[:, :, None, :]
        dist_ei = jnp.sqrt(jnp.sum(diff_ei * diff_ei, axis=-1))

        def mo(dist, coeff):
            ao = jnp.exp(-(dist[..., None] ** 2) * v['alpha'])
            return ao.reshape(ao.shape[:-2] + (N_AO,)) @ coeff

        mo_up = mo(dist_ei[:, :N_UP, :], v['mo_coeff_up'])
        mo_dn = mo(dist_ei[:, N_UP:, :], v['mo_coeff_dn'])
        # orbital gather as one-hot einsum (E_* precomputed host-side)
        sel_up = jnp.einsum('bim,dkm->bdik', mo_up, v['E_up'])
        sel_dn = jnp.einsum('bim,dkm->bdik', mo_dn, v['E_dn'])
        m_up = jnp.concatenate(
            [sel_up, jnp.zeros(sel_up.shape[:-1] + (N_DN,), sel_up.dtype)], -1)
        m_dn = jnp.concatenate(
            [jnp.zeros(sel_dn.shape[:-1] + (N_UP,), sel_dn.dtype), sel_dn], -1)
        m_up = m_up * v['ci_sc']

        def bf(h, W0, b0, W1, b1):
            y = jnp.tanh(h @ W0 + b0) @ W1 + b1
            y = y.reshape(y.shape[:-1] + (N_DETS, N_ORB))
            return jnp.swapaxes(y, -3, -2)

        m_up = m_up * bf(h_el[:, :N_UP, :], v['W_bf_up0'], v['b_bf_up0'],
                         v['W_bf_up1'], v['b_bf_up1'])
        m_dn = m_dn * bf(h_el[:, N_DN:, :], v['W_bf_dn0'], v['b_bf_dn0'],
                         v['W_bf_dn1'], v['b_bf_dn1'])
        return m_up, m_dn

    jf = jax.jit(fn, out_shardings=(shard_b, shard_b))
    return jf, shard_b, repl


def _host_precompute(ins):
    f32 = np.float32
    idx_up = np.asarray(ins['idx_up']).astype(np.int64)
    idx_dn = np.asarray(ins['idx_dn']).astype(np.int64)
    E_up = np.zeros((N_DETS, N_UP, N_MO), f32)
    E_dn = np.zeros((N_DETS, N_DN, N_MO), f32)
    r_up = np.arange(N_UP); r_dn = np.arange(N_DN)
    for d in range(N_DETS):
        E_up[d, r_up, idx_up[d]] = 1.0
        E_dn[d, r_dn, idx_dn[d]] = 1.0
    ciw = np.abs(ins['ci_weights'])[:, None, None] ** f32(1.0 / N_UP)
    sgn = np.concatenate([np.sign(ins['ci_weights'])[:, None, None],
                          np.ones((N_DETS, 1, N_ORB - 1), f32)], axis=-1)
    return E_up, E_dn, (ciw * sgn).astype(f32)


def _run_device(ins):
    global _DEVICE_FN
    import jax
    if _DEVICE_FN is None:
        _DEVICE_FN = _build_device_fn()
    jf, shard_b, repl = _DEVICE_FN

    E_up, E_dn, ci_sc = _host_precompute(ins)
    v = {k: np.require(ins[k], dtype=np.float32, requirements='C')
         for k in ins if k not in ('idx_up', 'idx_dn')}
    v['E_up'] = E_up; v['E_dn'] = E_dn; v['ci_sc'] = ci_sc
    # one batched pytree transfer — fewer round-trips over the axon tunnel
    vp = jax.device_put(v, {k: (shard_b if k in _BATCHED else repl)
                            for k in v})
    m_up, m_dn = jax.block_until_ready(jf(vp))
    return (np.asarray(m_up, dtype=np.float32),
            np.asarray(m_dn, dtype=np.float32))


# ---------------------------------------------------------------- numpy path
def _np_shift(h_el, pair, diff, dist, W0, b0, W1):
    u = h_el @ W0[:D] + b0
    v = pair @ W0[D:]
    z = np.tanh(u[:, :, None, :] + v)
    s = z @ W1
    wgt = s / (1.0 + dist[..., None] ** 3)
    return np.sum(wgt * diff, axis=-2)


def _run_numpy(ins):
    g = lambda k: np.asarray(ins[k]).astype(np.float32)
    h_el = g('h_el')
    s_el = _np_shift(h_el, g('h_el_el'), g('diff_el_el'), g('dist_el_el'),
                     g('W_shift_el0'), g('b_shift_el0'), g('W_shift_el1'))
    s_ion = _np_shift(h_el, g('h_el_ion'), g('diff_el_ion'), g('dist_el_ion'),
                      g('W_shift_ion0'), g('b_shift_ion0'), g('W_shift_ion1'))
    ls = g('decay_scale') / np.tanh(g('h_ion') @ g('W_decay')
                                    + g('b_decay'))[..., 0]
    decay = np.prod(np.tanh((g('dist_el_ion') / ls) ** 2), axis=-1)
    shift = (s_el + s_ion) * decay[..., None]
    diff_ei = g('diff_el_ion') + shift[:, :, None, :]
    dist_ei = np.sqrt(np.sum(diff_ei * diff_ei, axis=-1))

    alpha = g('alpha')

    def mo(dist, coeff):
        ao = np.exp(-(dist[..., None] ** 2) * alpha)
        return ao.reshape(ao.shape[:-2] + (N_AO,)) @ coeff

    mo_up = mo(dist_ei[:, :N_UP, :], g('mo_coeff_up'))
    mo_dn = mo(dist_ei[:, N_UP:, :], g('mo_coeff_dn'))
    idx_up = np.asarray(ins['idx_up'], dtype=np.int64)
    idx_dn = np.asarray(ins['idx_dn'], dtype=np.int64)
    sel_up = np.moveaxis(mo_up[..., idx_up], -2, -3)
    sel_dn = np.moveaxis(mo_dn[..., idx_dn], -2, -3)
    m_up = np.concatenate(
        [sel_up, np.zeros(sel_up.shape[:-1] + (N_DN,), sel_up.dtype)], -1)
    m_dn = np.concatenate(
        [np.zeros(sel_dn.shape[:-1] + (N_UP,), sel_dn.dtype), sel_dn], -1)
    _, _, ci_sc = _host_precompute(ins)
    m_up = m_up * ci_sc

    def bf(h, W0, b0, W1, b1):
        y = np.tanh(h @ W0 + b0) @ W1 + b1
        y = y.reshape(y.shape[:-1] + (N_DETS, N_ORB))
        return np.swapaxes(y, -3, -2)

    m_up = m_up * bf(h_el[:, :N_UP, :], g('W_bf_up0'), g('b_bf_up0'),
                     g('W_bf_up1'), g('b_bf_up1'))
    m_dn = m_dn * bf(h_el[:, N_DN:, :], g('W_bf_dn0'), g('b_bf_dn0'),
                     g('W_bf_dn1'), g('b_bf_dn1'))
    return m_up.astype(np.float32), m_dn.astype(np.float32)


def kernel(**inputs):
    global _DEVICE_INIT_FAILED
    if not _DEVICE_INIT_FAILED:
        try:
            return _run_device(inputs)
        except Exception:
            _DEVICE_INIT_FAILED = True
    return _run_numpy(inputs)
